# revision 13
# baseline (speedup 1.0000x reference)
"""Trainium2 Bass kernel for nn_Decoder_35905926595425.

Distributed over 8 NeuronCores:
  - attention (the dominant matmul over [S*B, E] @ [E, D]) sharded over S
  - W_out / logits sharded over V (4000 vocab columns per core)
  - LSTM gate matmuls sharded over D (256-padded-row slice per core)
  - the small serial chain (h_avg -> h0/c0 -> qpart) contraction-sharded
    with ReduceScatter/AllGather/AllReduce collectives
  - embedding gathers (emb[E_tm1], emb[argmax(y_tm1)]) done on host during
    input sharding (only 128 rows of the 64MB table are needed)

Matmuls run in bf16 (fp32 accumulation in PSUM); elementwise/softmax math in
fp32.  All cores execute one SPMD program; per-core behaviour differs only
through the input arrays bound to each core.
"""

import numpy as np
import ml_dtypes

import concourse.bass as bass
import concourse.mybir as mybir
import concourse.tile as tile
from concourse import bacc
from concourse.bass_utils import run_bass_kernel_spmd
from concourse.masks import make_identity

F32 = mybir.dt.float32
F32R = mybir.dt.float32r
BF16 = mybir.dt.bfloat16
AF = mybir.ActivationFunctionType
ALU = mybir.AluOpType
bf16 = ml_dtypes.bfloat16

# problem dims
S, B, E, D, W, V = 196, 64, 1024, 1800, 512, 32000
NCORES = 8
VS = V // NCORES            # 4000 vocab per core
NVC = 8                     # v chunks per core
VC = VS // NVC              # 500 (each padded to one 512-f32 PSUM bank)
DP = 2048                   # D padded to 16 chunks of 128
NDP = DP // 128             # 16
DSL = DP // NCORES          # 256-row D slice per core (2 chunks)
SCNT = [25, 25, 25, 25, 24, 24, 24, 24]
SOFF = np.cumsum([0] + SCNT).tolist()
SLOT = 25                   # padded s slots per core
SBF = 1664                  # 13*128 >= SLOT*64
NSB = SBF // 128            # 13
SALL = NCORES * SLOT * B    # 12800 gathered score entries
SROW = NCORES * SLOT        # 200 gathered s rows
XK = W + E + 128            # 1664 x rows: [emb 512 | ctx 1024 | ones 128]
NXK = XK // 128             # 13
AK = 1920 + 128 + W + E     # 3584 actT rows: [h 1920 | ones 128 | y 512 | c 1024]
NAK = AK // 128             # 28
AK_H = list(range(15))      # h_tT chunks
AK_LATE = list(range(15, 28))  # ones + y + ctx chunks (runnable before h_t)
D_H0, D_H1 = 904, 896       # attention d halves (bank-aligned splits)


def _bf(x):
    return np.ascontiguousarray(x.astype(bf16))


def _f32(x):
    return np.ascontiguousarray(x.astype(np.float32))


def _prep_inputs(E_tm1, y_tm1, h, emb, W_ih, b_ih, b_hh, W_hh, W_out, b_out,
                 Wh1, bh1, Wh2, bh2, Wc1, bc1, Wc2, bc2, Wa1, ba1, Wa2, ba2,
                 Wb, bb):
    """Build the per-core input dict list (host-side sharding/layout)."""
    h = np.asarray(h, np.float32)
    emb = np.asarray(emb, np.float32)
    y_tm1 = np.asarray(y_tm1, np.float32)

    x_emb = emb[np.asarray(E_tm1).astype(np.int64)]          # [B, W]
    y_emb = emb[np.argmax(y_tm1, axis=1)]                    # [B, W]
    embT_x = _bf(x_emb.T)                                    # [W, B]
    yembT = _bf(y_emb.T)                                     # [W, B]

    # attention weights (replicated)
    wa1hT = _bf(Wa1[:, D:].T)                                # [E, D]
    wa2rep = _bf(np.repeat(np.asarray(Wa2, np.float32), 128, axis=0))  # [128, D]

    # chain weights, transposed + padded to DP on the d axes
    wh1T = np.zeros((E, DP), np.float32); wh1T[:, :D] = np.asarray(Wh1).T / S
    wc1T = np.zeros((E, DP), np.float32); wc1T[:, :D] = np.asarray(Wc1).T / S
    wh2T = np.zeros((DP, DP), np.float32); wh2T[:D, :D] = np.asarray(Wh2).T
    wc2T = np.zeros((DP, DP), np.float32); wc2T[:D, :D] = np.asarray(Wc2).T
    wa1qT = np.zeros((DP, D), np.float32); wa1qT[:D, :] = np.asarray(Wa1)[:, :D].T
    wbT = np.zeros((DP, 1), np.float32); wbT[:D, 0] = np.asarray(Wb)[0]
    bh1p = np.zeros(DP, np.float32); bh1p[:D] = bh1
    bc1p = np.zeros(DP, np.float32); bc1p[:D] = bc1
    bh2p = np.zeros(DP, np.float32); bh2p[:D] = bh2
    bc2p = np.zeros(DP, np.float32); bc2p[:D] = bc2

    # LSTM weights: per-gate column slices of W_ih.T / W_hh.T (+ bias row)
    wihT_base = np.asarray(W_ih, np.float32).T               # [W+E, 4D]
    whhT_base = np.asarray(W_hh, np.float32).T               # [D, 4D]
    bias_ifgo = np.asarray(b_ih, np.float32) + np.asarray(b_hh, np.float32)

    wo = np.asarray(W_out, np.float32)                       # [V, 1800+512+1024]
    bo = np.asarray(b_out, np.float32)

    mask1 = np.zeros((128, B), np.float32)
    mask1[np.arange(128), np.arange(128) % B] = 1.0

    in_maps = []
    for c in range(NCORES):
        cnt = SCNT[c]
        hs = h[SOFF[c]:SOFF[c] + cnt].reshape(cnt * B, E)
        h_pad = np.zeros((SBF, E), np.float32)
        h_pad[:cnt * B] = hs

        smask = np.zeros((128, 1), np.float32)
        if c < 4:
            smask[64:, 0] = -1e30
        else:
            smask[:, 0] = -1e30

        rs, re = DSL * c, DSL * (c + 1)
        wih_sl = np.zeros((XK, 4 * DSL), np.float32)
        whh_sl = np.zeros((DP, 4 * DSL), np.float32)
        for g in range(4):
            ge = min(re, D)
            n = ge - rs
            if n > 0:
                wih_sl[:W + E, g * DSL:g * DSL + n] = \
                    wihT_base[:, g * D + rs:g * D + ge]
                wih_sl[W + E, g * DSL:g * DSL + n] = bias_ifgo[g * D + rs:g * D + ge]
                whh_sl[:D, g * DSL:g * DSL + n] = whhT_base[:, g * D + rs:g * D + ge]

        vs, ve = VS * c, VS * (c + 1)
        woT = np.zeros((AK, VS), np.float32)
        woT[0:D, :] = wo[vs:ve, 0:D].T                       # h section
        woT[1920, :] = bo[vs:ve]                             # ones row -> bias
        woT[2048:2048 + W, :] = wo[vs:ve, D:D + W].T         # y section
        woT[2560:2560 + E, :] = wo[vs:ve, D + W:].T          # ctx section

        m = {
            "h_sb": _f32(h_pad),
            "hT": _bf(h_pad.T),
            "wa1hT": wa1hT,
            "wa2rep": wa2rep,
            "mask1": mask1,
            "smask": smask,
            "embT_x": embT_x,
            "yembT": yembT,
            "wh1T_sl": _bf(wh1T[128 * c:128 * (c + 1)]),
            "wc1T_sl": _bf(wc1T[128 * c:128 * (c + 1)]),
            "bh1_sl": _f32(bh1p[rs:re].reshape(2, 128).T),
            "bc1_sl": _f32(bc1p[rs:re].reshape(2, 128).T),
            "wh2T_sl": _bf(wh2T[rs:re]),
            "wc2T_sl": _bf(wc2T[rs:re]),
            "bh2_sl": _f32(bh2p[rs:re].reshape(2, 128).T),
            "bc2_sl": _f32(bc2p[rs:re].reshape(2, 128).T),
            "wa1qT_sl": _bf(wa1qT[rs:re]),
            "wbT_sl": _bf(wbT[rs:re]),
            "ba1_64": _f32(np.repeat(np.asarray(ba1, np.float32).reshape(1, D),
                                     B, axis=0)),
            "bb_64": _f32(np.full((B, 1), np.asarray(bb, np.float32).ravel()[0])),
            "wihT_sl": _bf(wih_sl),
            "whhT_sl": _bf(whh_sl),
            "woT": _bf(woT),
        }
        in_maps.append(m)
    return in_maps


_NC_CACHE = {}


def _build_nc():
    if "nc" in _NC_CACHE:
        return _NC_CACHE["nc"]
    nc = bacc.Bacc("TRN2", target_bir_lowering=False, debug=False,
                   num_devices=NCORES)
    di = {}

    def inp(name, shape, dt=F32):
        di[name] = nc.dram_tensor(name, list(shape), dt, kind="ExternalInput")
        return di[name]

    inp("h_sb", (SBF, E)); inp("hT", (E, SBF), BF16)
    inp("wa1hT", (E, D), BF16); inp("wa2rep", (128, D), BF16)
    inp("mask1", (128, B), F32R); inp("smask", (128, 1))
    inp("embT_x", (W, B), BF16); inp("yembT", (W, B), BF16)
    inp("wh1T_sl", (128, DP), BF16); inp("wc1T_sl", (128, DP), BF16)
    inp("bh1_sl", (128, 2)); inp("bc1_sl", (128, 2))
    inp("wh2T_sl", (DSL, DP), BF16); inp("wc2T_sl", (DSL, DP), BF16)
    inp("bh2_sl", (128, 2)); inp("bc2_sl", (128, 2))
    inp("wa1qT_sl", (DSL, D), BF16); inp("wbT_sl", (DSL, 1), BF16)
    inp("ba1_64", (B, D)); inp("bb_64", (B, 1))
    inp("wihT_sl", (XK, 4 * DSL), BF16); inp("whhT_sl", (DP, 4 * DSL), BF16)
    inp("woT", (AK, VS), BF16)

    logits_out = nc.dram_tensor("logits_out", [B, VS], F32, kind="ExternalOutput")
    ht_out = nc.dram_tensor("ht_out", [DSL, B], F32, kind="ExternalOutput")
    ct_out = nc.dram_tensor("ct_out", [DSL, B], F32, kind="ExternalOutput")
    aw_out = nc.dram_tensor("aw_out", [B, SROW], F32, kind="ExternalOutput")

    _emit(nc, di, logits_out, ht_out, ct_out, aw_out)
    nc.finalize()
    _NC_CACHE["nc"] = nc
    return nc


def _emit(nc, di, logits_out, ht_out, ct_out, aw_out):
    RG = [list(range(NCORES))]
    dma = nc.sync.dma_start

    with tile.TileContext(nc) as tc:
        with (
            tc.tile_pool(name="persist", bufs=1) as P,
            tc.tile_pool(name="dram", bufs=1, space="DRAM") as DR,
        ):
            # ---------- persistent small tiles ----------
            ident = P.tile([128, 128], F32)
            make_identity(nc, ident[:])

            mask1 = P.tile([128, B], F32R, tag="mask1")
            dma(mask1[:], di["mask1"][:, :])
            smask = P.tile([128, 1], F32, tag="smask")
            dma(smask[:], di["smask"][:, :])
            wa2rep = P.tile([128, D], BF16, tag="wa2rep")
            dma(wa2rep[:], di["wa2rep"][:, :])

            score = P.tile([128, NSB], F32)
            h0T_full = P.tile([128, NDP, B], BF16)
            c0T_sl = P.tile([128, 2, B], F32)
            h0T_sl = P.tile([128, 2, B], BF16)
            beta = P.tile([B, 1], F32)
            ctx_full = P.tile([B, E], F32)
            xT = P.tile([128, NXK, B], BF16)      # [emb | ctx | ones] columns

            # ---------- DRAM bounce buffers for collectives ----------
            rs1_in = DR.tile([E, B], F32, tag="rs1_in")
            rs1_out = DR.tile([128, B], F32, tag="rs1_out")
            rs2_in = DR.tile([2 * DP, B], F32, tag="rs2_in")
            rs2_out = DR.tile([2 * DSL, B], F32, tag="rs2_out")
            rs3_in = DR.tile([2 * DP, B], F32, tag="rs3_in")
            rs3_out = DR.tile([2 * DSL, B], F32, tag="rs3_out")
            ag3_in = DR.tile([DSL, B], F32, tag="ag3_in")
            ag3_out = DR.tile([DP, B], F32, tag="ag3_out")
            ar4_in = DR.tile([B, D + 1], F32, tag="ar4_in")
            ar4_out = DR.tile([B, D + 1], F32, tag="ar4_out")
            ags_in = DR.tile([SLOT * B], F32, tag="ags_in")
            ags_out = DR.tile([SALL], F32, tag="ags_out")
            ar5_in = DR.tile([B, E], F32, tag="ar5_in")
            ar5_out = DR.tile([B, E], F32, tag="ar5_out")
            ag6_in = DR.tile([DSL, B], F32, tag="ag6_in")
            ag6_out = DR.tile([DP, B], F32, tag="ag6_out")

            # ================= phase A: attention + chain =================
            with (
                tc.tile_pool(name="attnw", bufs=1) as AWP,
                tc.tile_pool(name="psA", bufs=2, space="PSUM") as PSA,
                tc.tile_pool(name="chainps", bufs=2, space="PSUM") as CPS,
                tc.tile_pool(name="chev", bufs=4) as CHE,
                tc.tile_pool(name="qps", bufs=1, space="PSUM") as QPS,
            ):
                qrep = AWP.tile([128, D], BF16)
                hT = AWP.tile([128, 8, SBF], BF16)
                for k in range(8):
                    dma(hT[:, k, :], di["hT"].ap().rearrange(
                        "(k p) n -> k p n", p=128)[k])
                wa1hT = AWP.tile([128, 8, D], BF16)
                for k in range(8):
                    dma(wa1hT[:, k, :], di["wa1hT"].ap().rearrange(
                        "(k p) n -> k p n", p=128)[k])

                # --- h sum over local s (in T layout) + ReduceScatter ---
                haT = AWP.tile([128, 8, B], F32)
                nc.vector.tensor_reduce(
                    haT[:], hT[:].rearrange("p k (s b) -> p k b s", b=B),
                    axis=mybir.AxisListType.X, op=ALU.add)
                for k in range(8):
                    dma(rs1_in[128 * k:128 * (k + 1), :], haT[:, k, :])
                nc.gpsimd.collective_compute(
                    "ReduceScatter", ALU.add, replica_groups=RG,
                    ins=[rs1_in[:].opt()], outs=[rs1_out[:].opt()])
                haT_sl = AWP.tile([128, B], BF16)
                haT_sl_f = AWP.tile([128, B], F32)
                dma(haT_sl_f[:], rs1_out[:, :])
                nc.vector.tensor_copy(haT_sl[:], haT_sl_f[:])

                # --- pre_h / pre_c partials (e-contraction-sharded) ---
                wh1T = AWP.tile([128, DP], BF16, tag="wh1T")
                dma(wh1T[:], di["wh1T_sl"][:, :])
                wc1T = AWP.tile([128, DP], BF16, tag="wc1T")
                dma(wc1T[:], di["wc1T_sl"][:, :])
                for half, wt in ((0, wh1T), (1, wc1T)):
                    for m in range(NDP):
                        ps = CPS.tile([128, B], F32, tag="chainps")
                        nc.tensor.matmul(ps[:], wt[:, 128 * m:128 * (m + 1)],
                                         haT_sl[:], start=True, stop=True)
                        ev = CHE.tile([128, B], F32, tag="chev")
                        nc.vector.tensor_copy(ev[:], ps[:])
                        blk = m // 2
                        ro = half * DSL + (m % 2) * 128
                        dma(rs2_in[512 * blk + ro:512 * blk + ro + 128, :], ev[:])
                nc.gpsimd.collective_compute(
                    "ReduceScatter", ALU.add, replica_groups=RG,
                    ins=[rs2_in[:].opt()], outs=[rs2_out[:].opt()])

                # --- relu(pre + b1) on own slice ---
                bh1 = AWP.tile([128, 2], F32, tag="bh1")
                dma(bh1[:], di["bh1_sl"][:, :])
                bc1 = AWP.tile([128, 2], F32, tag="bc1")
                dma(bc1[:], di["bc1_sl"][:, :])
                pre_sl = AWP.tile([128, 2, B], F32)
                prc_sl = AWP.tile([128, 2, B], F32)
                dma(pre_sl[:], rs2_out[:].rearrange("(k p) b -> p k b", p=128)[:, 0:2])
                dma(prc_sl[:], rs2_out[:].rearrange("(k p) b -> p k b", p=128)[:, 2:4])
                preh_bf = AWP.tile([128, 2, B], BF16)
                prec_bf = AWP.tile([128, 2, B], BF16)
                for k in range(2):
                    nc.scalar.activation(preh_bf[:, k, :], pre_sl[:, k, :],
                                         AF.Relu, bias=bh1[:, k:k + 1])
                    nc.scalar.activation(prec_bf[:, k, :], prc_sl[:, k, :],
                                         AF.Relu, bias=bc1[:, k:k + 1])

                # --- h0T / c0T partials (d'-contraction-sharded) ---
                wh2T = AWP.tile([128, 2, DP], BF16)
                dma(wh2T[:], di["wh2T_sl"].ap().rearrange("(k p) n -> p k n", p=128))
                wc2T = AWP.tile([128, 2, DP], BF16)
                dma(wc2T[:], di["wc2T_sl"].ap().rearrange("(k p) n -> p k n", p=128))
                for half, wt, pr in ((0, wh2T, preh_bf),
                                     (1, wc2T, prec_bf)):
                    for m in range(NDP):
                        ps = CPS.tile([128, B], F32, tag="chainps")
                        for k in range(2):
                            nc.tensor.matmul(ps[:], wt[:, k, 128 * m:128 * (m + 1)],
                                             pr[:, k, :], start=(k == 0),
                                             stop=(k == 1))
                        ev = CHE.tile([128, B], F32, tag="chev")
                        nc.vector.tensor_copy(ev[:], ps[:])
                        blk = m // 2
                        ro = half * DSL + (m % 2) * 128
                        dma(rs3_in[512 * blk + ro:512 * blk + ro + 128, :], ev[:])
                nc.gpsimd.collective_compute(
                    "ReduceScatter", ALU.add, replica_groups=RG,
                    ins=[rs3_in[:].opt()], outs=[rs3_out[:].opt()])

                h0T_sl_f = AWP.tile([128, 2, B], F32)
                dma(h0T_sl_f[:], rs3_out[:].rearrange("(k p) b -> p k b", p=128)[:, 0:2])
                dma(c0T_sl[:], rs3_out[:].rearrange("(k p) b -> p k b", p=128)[:, 2:4])
                bh2 = AWP.tile([128, 2], F32, tag="bh2")
                dma(bh2[:], di["bh2_sl"][:, :])
                bc2 = AWP.tile([128, 2], F32, tag="bc2")
                dma(bc2[:], di["bc2_sl"][:, :])
                for k in range(2):
                    nc.vector.tensor_scalar_add(h0T_sl_f[:, k, :],
                                                h0T_sl_f[:, k, :],
                                                bh2[:, k:k + 1])
                    nc.vector.tensor_scalar_add(c0T_sl[:, k, :],
                                                c0T_sl[:, k, :],
                                                bc2[:, k:k + 1])
                nc.vector.tensor_copy(h0T_sl[:], h0T_sl_f[:])
                for k in range(2):
                    dma(ag3_in[128 * k:128 * (k + 1), :], h0T_sl_f[:, k, :])
                nc.gpsimd.collective_compute(
                    "AllGather", ALU.bypass, replica_groups=RG,
                    ins=[ag3_in[:].opt()], outs=[ag3_out[:].opt()])
                h0T_full_f = AWP.tile([128, NDP, B], F32)
                dma(h0T_full_f[:], ag3_out[:].rearrange("(k p) b -> p k b", p=128))
                nc.vector.tensor_copy(h0T_full[:], h0T_full_f[:])

                # --- qpart + beta partials, AllReduce ---
                wa1qT = AWP.tile([128, 2, D], BF16)
                dma(wa1qT[:], di["wa1qT_sl"].ap().rearrange("(k p) n -> p k n", p=128))
                wbT = AWP.tile([128, 2, 1], BF16)
                dma(wbT[:], di["wbT_sl"].ap().rearrange("(k p) n -> p k n", p=128))
                qp_sb = AWP.tile([B, D], F32, tag="qpbuf")
                for h0_, hw in ((0, D_H0), (1, D_H1)):
                    off = h0_ * D_H0
                    ps = QPS.tile([B, D_H0], F32, tag="qps")
                    nsizes = [512, hw - 512]
                    for k in range(2):
                        no = 0
                        for nn in nsizes:
                            nc.tensor.matmul(
                                ps[:, no:no + nn], h0T_sl[:, k, :],
                                wa1qT[:, k, off + no:off + no + nn],
                                start=(k == 0), stop=(k == 1))
                            no += nn
                    nc.scalar.copy(qp_sb[:, off:off + hw], ps[:, 0:hw])
                psb = QPS.tile([B, 1], F32, tag="qps")
                for k in range(2):
                    nc.tensor.matmul(psb[:], h0T_sl[:, k, :], wbT[:, k, :],
                                     start=(k == 0), stop=(k == 1))
                bpre_sb = AWP.tile([B, 1], F32)
                nc.vector.tensor_copy(bpre_sb[:], psb[:])
                dma(ar4_in[:, 0:D], qp_sb[:])
                dma(ar4_in[:, D:D + 1], bpre_sb[:])
                nc.gpsimd.collective_compute(
                    "AllReduce", ALU.add, replica_groups=RG,
                    ins=[ar4_in[:].opt()], outs=[ar4_out[:].opt()])
                with tc.tile_pool(name="qtmp", bufs=1) as QT:
                    qp_full = QT.tile([B, D], F32)
                    bpre = QT.tile([B, 1], F32)
                    ba1_64 = QT.tile([B, D], F32)
                    bb_64 = QT.tile([B, 1], F32)
                    dma(qp_full[:], ar4_out[:, 0:D])
                    dma(bpre[:], ar4_out[:, D:D + 1])
                    dma(ba1_64[:], di["ba1_64"][:, :])
                    dma(bb_64[:], di["bb_64"][:, :])
                    nc.vector.tensor_tensor(bpre[:], bpre[:], bb_64[:],
                                            op=ALU.add)
                    nc.scalar.activation(beta[:], bpre[:], AF.Sigmoid)
                    nc.vector.tensor_tensor(qrep[0:64, :], qp_full[:],
                                            ba1_64[:], op=ALU.add)
                    nc.vector.tensor_tensor(qrep[64:128, :], qp_full[:],
                                            ba1_64[:], op=ALU.add)

                # --- big attention matmul: u0[sb, d] = hT.T @ wa1hT ---
                u0_tiles = []
                with tc.tile_pool(name="u0p", bufs=10) as U0P:
                    for j in range(NSB):
                        u0j = U0P.tile([128, D], BF16, tag="u0")
                        u0_tiles.append(u0j)
                        for off, hw in ((0, D_H0), (D_H0, D_H1)):
                            ps = PSA.tile([128, D_H0], F32, tag="psA")
                            nsizes = [512, hw - 512]
                            for k in range(8):
                                no = 0
                                for nn in nsizes:
                                    nc.tensor.matmul(
                                        ps[:, no:no + nn],
                                        hT[:, k, 128 * j:128 * (j + 1)],
                                        wa1hT[:, k, off + no:off + no + nn],
                                        start=(k == 0), stop=(k == 7))
                                    no += nn
                            nc.scalar.copy(u0j[:, off:off + hw], ps[:, 0:hw])

                    # --- scores: relu(u0 + qrep) . wa2 ---
                    with tc.tile_pool(name="uscr", bufs=2) as USC:
                        for j in range(NSB):
                            u1 = USC.tile([128, D], BF16, tag="u1")
                            nc.vector.tensor_tensor(u1[:], u0_tiles[j][:],
                                                    qrep[:], op=ALU.add)
                            nc.vector.scalar_tensor_tensor(
                                u1[:], u1[:], 0.0, wa2rep[:],
                                op0=ALU.max, op1=ALU.mult,
                                accum_out=score[:, j:j + 1])

            # ================= phase B: softmax + ctx + lstm + logits ====
            nc.vector.tensor_scalar_add(score[:, NSB - 1:NSB],
                                        score[:, NSB - 1:NSB], smask[:])
            dma(ags_in[0:1536].rearrange("(j p) -> p j", p=128), score[:, 0:12])
            dma(ags_in[1536:1600].rearrange("(p n) -> p n", n=1),
                score[0:64, 12:13])
            nc.gpsimd.collective_compute(
                "AllGather", ALU.bypass, replica_groups=RG,
                ins=[ags_in[:].opt()], outs=[ags_out[:].opt()])

            with (
                tc.tile_pool(name="late", bufs=1) as L,
                tc.tile_pool(name="wot", bufs=5) as WOT,
            ):
              with (
                tc.tile_pool(name="pssm", bufs=1, space="PSUM") as PSM,
                tc.tile_pool(name="psctx", bufs=1, space="PSUM") as PCX,
                tc.tile_pool(name="whp", bufs=2) as WHP,
              ):
                # ---- softmax stats over gathered scores ----
                st_a = L.tile([128, B], F32)
                st_b = L.tile([128, B], F32)
                nc.vector.memset(st_b[:], -1e30)
                sc_all = ags_out[:].rearrange("(r b) -> r b", b=B)
                dma(st_a[:], sc_all[0:128])
                dma(st_b[0:72, :], sc_all[128:200])
                psT = PSM.tile([B, 256], F32, tag="pssm")
                nc.tensor.transpose(psT[:, 0:128], st_a[:], ident[:])
                nc.tensor.transpose(psT[:, 128:256], st_b[:], ident[:])
                nmax = L.tile([B, 1], F32)
                nc.vector.tensor_reduce(nmax[:], psT[:],
                                        axis=mybir.AxisListType.X, op=ALU.max)
                nc.vector.tensor_scalar_mul(nmax[:], nmax[:], -1.0)
                expw = L.tile([B, 256], F32)
                sumexp = L.tile([B, 1], F32)
                nc.scalar.activation(expw[:], psT[:], AF.Exp, bias=nmax[:],
                                     accum_out=sumexp[:])
                recip = L.tile([B, 1], F32)
                nc.vector.reciprocal(recip[:], sumexp[:])
                awf = L.tile([B, SROW], F32)
                nc.vector.tensor_scalar_mul(awf[:], expw[:, 0:SROW], recip[:])
                dma(aw_out[:, :], awf[:])

                # ---- aw in sb layout ----
                nm2 = L.tile([128, 1], F32)
                rc2 = L.tile([128, 1], F32)
                nc.vector.tensor_copy(nm2[0:64, :], nmax[:])
                nc.vector.tensor_copy(nm2[64:128, :], nmax[:])
                nc.vector.tensor_copy(rc2[0:64, :], recip[:])
                nc.vector.tensor_copy(rc2[64:128, :], recip[:])
                aw13 = L.tile([128, NSB], F32)
                nc.scalar.activation(aw13[:], score[:], AF.Exp, bias=nm2[:])
                nc.vector.tensor_scalar_mul(aw13[:], aw13[:], rc2[:])

                # ---- ctx partial = sum_s aw * h (h streamed per chunk) ----
                pctx = PCX.tile([B, 2, 512], F32, tag="psctx")
                for j in range(NSB):
                    hch = WHP.tile([128, E], F32, tag="hch")
                    dma(hch[:], di["h_sb"].ap().rearrange(
                        "(j p) e -> j p e", p=128)[j])
                    wh_ = WHP.tile([128, E], F32R, tag="whp")
                    nc.vector.tensor_scalar_mul(wh_[:], hch[:],
                                                aw13[:, j:j + 1])
                    for v in range(2):
                        nc.tensor.matmul(pctx[:, v, :],
                                         mask1[:],
                                         wh_[:, 512 * v:512 * (v + 1)],
                                         start=(j == 0), stop=(j == NSB - 1))
                ctx_p = L.tile([B, E], F32)
                nc.scalar.copy(ctx_p[:], pctx[:].rearrange("b v n -> b (v n)"))
                dma(ar5_in[:, :], ctx_p[:])
                nc.gpsimd.collective_compute(
                    "AllReduce", ALU.add, replica_groups=RG,
                    ins=[ar5_in[:].opt()], outs=[ar5_out[:].opt()])
                dma(ctx_full[:], ar5_out[:, :])
                nc.vector.tensor_scalar_mul(ctx_full[:], ctx_full[:], beta[:])

                # ---- xT assembly: [emb | ctx | ones] ----
                dma(xT[:, 0:4, :],
                    di["embT_x"].ap().rearrange("(k p) b -> p k b", p=128))
                for j in range(8):
                    pst = PSM.tile([128, B], F32, tag="psxt")
                    nc.tensor.transpose(pst[:], ctx_full[:, 128 * j:128 * (j + 1)],
                                        ident[0:64, 0:64])
                    nc.vector.tensor_copy(xT[:, 4 + j, :], pst[:])
                nc.vector.memset(xT[:, 12, :], 0.0)
                nc.vector.memset(xT[0:1, 12, :], 1.0)

              with tc.tile_pool(name="gps", bufs=1, space="PSUM") as GPS:
                # ---- LSTM gates (own 256-row d slice, T layout) ----
                wih = L.tile([128, NXK, 4 * DSL], BF16)
                dma(wih[:], di["wihT_sl"].ap().rearrange("(k p) n -> p k n", p=128))
                whh = L.tile([128, NDP, 4 * DSL], BF16)
                dma(whh[:], di["whhT_sl"].ap().rearrange("(k p) n -> p k n", p=128))
                ht_sb = L.tile([128, 2, B], F32)
                ct_sb = L.tile([128, 2, B], F32)
                for m in range(2):
                    gp = GPS.tile([128, 4, B], F32, tag="gps")
                    for g in range(4):
                        co = g * DSL + m * 128
                        for k in range(NXK):
                            nc.tensor.matmul(gp[:, g, :],
                                             wih[:, k, co:co + 128], xT[:, k, :],
                                             start=(k == 0), stop=False)
                        for k in range(NDP):
                            nc.tensor.matmul(gp[:, g, :],
                                             whh[:, k, co:co + 128],
                                             h0T_full[:, k, :],
                                             start=False, stop=(k == NDP - 1))
                    sig_i = L.tile([128, B], F32, tag="lse0")
                    sig_f = L.tile([128, B], F32, tag="lse1")
                    tah_g = L.tile([128, B], F32, tag="lse2")
                    sig_o = L.tile([128, B], F32, tag="lse3")
                    nc.scalar.activation(sig_i[:], gp[:, 0, :], AF.Sigmoid)
                    nc.scalar.activation(sig_f[:], gp[:, 1, :], AF.Sigmoid)
                    nc.scalar.activation(tah_g[:], gp[:, 2, :], AF.Tanh)
                    nc.scalar.activation(sig_o[:], gp[:, 3, :], AF.Sigmoid)
                    nc.vector.tensor_tensor(sig_f[:], sig_f[:], c0T_sl[:, m, :],
                                            op=ALU.mult)
                    nc.vector.tensor_tensor(sig_i[:], sig_i[:], tah_g[:],
                                            op=ALU.mult)
                    nc.vector.tensor_tensor(ct_sb[:, m, :], sig_f[:], sig_i[:],
                                            op=ALU.add)
                    nc.scalar.activation(tah_g[:], ct_sb[:, m, :], AF.Tanh)
                    nc.vector.tensor_tensor(ht_sb[:, m, :], sig_o[:], tah_g[:],
                                            op=ALU.mult)
                for m in range(2):
                    dma(ct_out[128 * m:128 * (m + 1), :], ct_sb[:, m, :])
                    dma(ht_out[128 * m:128 * (m + 1), :], ht_sb[:, m, :])
                    dma(ag6_in[128 * m:128 * (m + 1), :], ht_sb[:, m, :])
              nc.gpsimd.collective_compute(
                  "AllGather", ALU.bypass, replica_groups=RG,
                  ins=[ag6_in[:].opt()], outs=[ag6_out[:].opt()])

              with tc.tile_pool(name="pslg", bufs=1, space="PSUM") as PLG:
                # ---- logits part A (ones + y + ctx sections) ----
                woT_dram = di["woT"].ap().rearrange("(k p) v -> k p v", p=128)
                actT_oy = L.tile([128, 5, B], BF16)
                # zero via a score-dependent multiply: delays the logits PSUM
                # allocation until the attention-phase PSUM has been released
                nc.vector.tensor_scalar_mul(
                    actT_oy[:, 0, :],
                    score[:, NSB - 1:NSB].broadcast_to([128, B]), 0.0)
                nc.vector.memset(actT_oy[0:1, 0, :], 1.0)
                dma(actT_oy[:, 1:5, :],
                    di["yembT"].ap().rearrange("(k p) b -> p k b", p=128))

                def act_chunk(k):
                    if k < 15:
                        return h_tT_bf[:, k, :]
                    if k < 20:
                        return actT_oy[:, k - 15, :]
                    return xT[:, k - 16, :]

                lacc = L.tile([B, NVC, VC], F32)
                psa = PLG.tile([B, NVC, 512], F32, tag="pslg")
                for ki, k in enumerate(AK_LATE):
                    wt = WOT.tile([128, VS], BF16, tag="wot")
                    dma(wt[:], woT_dram[k])
                    for v in range(NVC):
                        nc.tensor.matmul(psa[:, v, 0:VC], act_chunk(k),
                                         wt[:, VC * v:VC * (v + 1)],
                                         start=(ki == 0),
                                         stop=(ki == len(AK_LATE) - 1))
                nc.vector.tensor_copy(lacc[:], psa[:, :, 0:VC])

                h_tT_f = L.tile([128, 15, B], F32)
                dma(h_tT_f[:], ag6_out[:].rearrange("(k p) b -> p k b", p=128)[:, 0:15])
                h_tT_bf = L.tile([128, 15, B], BF16)
                nc.vector.tensor_copy(h_tT_bf[:], h_tT_f[:])

                # ---- logits part B (h section) + combine ----
                psb_ = PLG.tile([B, NVC, 512], F32, tag="pslg")
                for ki, k in enumerate(AK_H):
                    wt = WOT.tile([128, VS], BF16, tag="wot")
                    dma(wt[:], woT_dram[k])
                    for v in range(NVC):
                        nc.tensor.matmul(psb_[:, v, 0:VC], h_tT_bf[:, k, :],
                                         wt[:, VC * v:VC * (v + 1)],
                                         start=(ki == 0),
                                         stop=(ki == len(AK_H) - 1))
                nc.vector.tensor_tensor(lacc[:], psb_[:, :, 0:VC], lacc[:],
                                        op=ALU.add)
                dma(logits_out[:, :], lacc[:].rearrange("b v n -> b (v n)"))


RUN_KWARGS = {}
LAST_RESULTS = None


def kernel(**inputs):
    global LAST_RESULTS
    in_maps = _prep_inputs(**inputs)
    nc = _build_nc()
    res = run_bass_kernel_spmd(nc, in_maps, core_ids=list(range(NCORES)),
                               **{"trace": False, **RUN_KWARGS})
    LAST_RESULTS = res
    r = res.results

    logits = np.concatenate([r[c]["logits_out"] for c in range(NCORES)], axis=1)
    htT = np.concatenate([r[c]["ht_out"] for c in range(NCORES)], axis=0)[:D]
    ctT = np.concatenate([r[c]["ct_out"] for c in range(NCORES)], axis=0)[:D]
    h_t = np.ascontiguousarray(htT.T)
    c_t = np.ascontiguousarray(ctT.T)

    awT = r[0]["aw_out"].T                                   # [200, 64]
    valid = [SLOT * c + i for c in range(NCORES) for i in range(SCNT[c])]
    aw = np.ascontiguousarray(awT[valid])[:, :, None]        # [196, 64, 1]

    return (logits.astype(np.float32), h_t.astype(np.float32),
            c_t.astype(np.float32), aw.astype(np.float32))
